# revision 1
# baseline (speedup 1.0000x reference)
"""LATTE GNN forward on 8 Trainium2 NeuronCores.

Math: the reference's per-edge message is v[dst] (the destination node's own
projected feature), and segment-softmax weights over each destination's
incoming edges sum to exactly 1.  Hence the edge aggregation reduces to
    h_m[n] = v[n] * mask_m[n],   mask_m[n] = [node n has >=1 incoming edge in rel m]
and the whole module collapses to
    v      = feat @ Wr + br                       [N, 256]
    vl[n,h]= v[n,h,:] . rel_attn_l[h]             (= feat @ (Wr @ RLbd) + br.RLbd)
    vr[n,h]= v[n,h,:] . rel_attn_r[h]
    logit[n,r,h] = lrelu(vl + mask_r * vr)
    beta   = softmax over h (axis=2 of [N,M+1,H] in the reference!)
    s[n,h] = sum_r mask_r[n] * beta[n,r,h]        (mask_3 = 1)
    out    = relu(LN(v * s) * gamma + ln_beta)
Node-sharded across 8 cores (rows 6250/core, padded to 6272 = 49*128).
Edge structure enters only through the per-node masks (host bincount).
"""

import numpy as np

N, D, H, C, M = 50000, 256, 4, 64, 3
NCORES = 8
RPC = N // NCORES          # 6250 rows per core
NT = 49                    # 128-row tiles per core
RPAD = NT * 128            # 6272
EPS = 1e-5

_CACHE = {}
LAST_RESULT = None         # BassKernelResults of the most recent run (for test.py)


def _build(trace=False):
    import concourse.bass as bass
    import concourse.mybir as mybir
    from concourse.tile import TileContext

    fp32 = mybir.dt.float32
    AF = mybir.ActivationFunctionType
    OP = mybir.AluOpType

    nc = bass.Bass()
    featT = nc.declare_dram_parameter("featT", [128, 2, RPAD], fp32, isOutput=False)
    constd = nc.declare_dram_parameter("constd", [128, 1628], fp32, isOutput=False)
    out = nc.declare_dram_parameter("out", [RPAD, 256], fp32, isOutput=True)

    with TileContext(nc) as tc:
        with (
            tc.tile_pool(name="const", bufs=1) as cpool,
            tc.tile_pool(name="ft", bufs=4) as ftpool,
            tc.tile_pool(name="small", bufs=4) as spool,
            tc.tile_pool(name="big", bufs=3) as bpool,
            tc.tile_pool(name="psv", bufs=2, space="PSUM") as pvpool,
            tc.tile_pool(name="pslv", bufs=2, space="PSUM") as plpool,
        ):
            const_sb = cpool.tile([128, 1628], fp32, tag="const")
            nc.gpsimd.dma_start(out=const_sb[:], in_=constd[:])
            # layout: [0:512) Wr k-chunks, [512:528) A k-chunks,
            # [528:784) gamma, [784:1040) beta,
            # row0 [1040:1304) biasrow, row0 [1304:1432) ones,
            # [1432:1628) per-tile masks (tile i -> [1432+4i, 1436+4i))
            gam_sb = const_sb[:, 528:784]
            bet_sb = const_sb[:, 784:1040]
            # dummy matmul: absorbs the const-DMA wait on PE so later
            # matmuls carry only their own ftT-DMA wait (1-wait ISA limit)
            dummy_ps = plpool.tile([128, 1], fp32, tag="lv")
            nc.tensor.matmul(dummy_ps[:], const_sb[0:1, 1304:1432],
                             const_sb[0:1, 1040:1041], start=True, stop=True)

            for i in range(NT):
                r0 = i * 128
                ftT = ftpool.tile([128, 2, 128], fp32, tag="ftT")
                nc.sync.dma_start(out=ftT[:], in_=featT[:, :, r0:r0 + 128])
                mk = const_sb[:, 1432 + 4 * i:1436 + 4 * i]

                # v = feat @ Wr + br    [128 rows, 256]
                v_ps = pvpool.tile([128, 256], fp32, tag="v")
                nc.tensor.matmul(v_ps[:], ftT[:, 0, :], const_sb[:, 0:256], start=True, stop=False)
                nc.tensor.matmul(v_ps[:], ftT[:, 1, :], const_sb[:, 256:512], start=False, stop=False)
                nc.tensor.matmul(v_ps[:], const_sb[0:1, 1304:1432],
                                 const_sb[0:1, 1040:1296], start=False, stop=True)
                # [vl | vr]   [128, 8]
                lv_ps = plpool.tile([128, 8], fp32, tag="lv")
                nc.tensor.matmul(lv_ps[:], ftT[:, 0, :], const_sb[:, 512:520], start=True, stop=False)
                nc.tensor.matmul(lv_ps[:], ftT[:, 1, :], const_sb[:, 520:528], start=False, stop=False)
                nc.tensor.matmul(lv_ps[:], const_sb[0:1, 1304:1432],
                                 const_sb[0:1, 1296:1304], start=False, stop=True)

                mk3 = mk.unsqueeze(2).broadcast_to((128, 4, 4))      # (r,h) r-major
                vl3 = lv_ps[:, 0:4].unsqueeze(1).broadcast_to((128, 4, 4))
                vr3 = lv_ps[:, 4:8].unsqueeze(1).broadcast_to((128, 4, 4))

                lg = spool.tile([128, 16], fp32, tag="lg")
                lg3 = lg[:].rearrange("p (r h) -> p r h", r=4)
                nc.vector.tensor_tensor(out=lg3, in0=mk3, in1=vr3, op=OP.mult)
                nc.vector.tensor_tensor(out=lg3, in0=lg3, in1=vl3, op=OP.add)
                lr = spool.tile([128, 16], fp32, tag="lr")
                # leaky_relu(x) = max(0.2*x, x)
                nc.vector.scalar_tensor_tensor(out=lr[:], in0=lg[:], scalar=0.2,
                                               in1=lg[:], op0=OP.mult, op1=OP.max)
                ext = spool.tile([128, 16], fp32, tag="ext")
                nc.scalar.activation(ext[:], lr[:], AF.Exp)
                ex3 = ext[:].rearrange("p (r h) -> p r h", r=4)
                den = spool.tile([128, 4], fp32, tag="den")
                nc.vector.tensor_reduce(out=den[:], in_=ex3, axis=mybir.AxisListType.X,
                                        op=OP.add)
                rden = spool.tile([128, 4], fp32, tag="rden")
                nc.vector.reciprocal(rden[:], den[:])
                mrd = spool.tile([128, 4], fp32, tag="mrd")
                nc.vector.tensor_tensor(out=mrd[:], in0=mk, in1=rden[:], op=OP.mult)
                wex = spool.tile([128, 16], fp32, tag="wex")
                wex3 = wex[:].rearrange("p (r h) -> p r h", r=4)
                nc.vector.tensor_tensor(out=wex3, in0=ex3,
                                        in1=mrd[:].unsqueeze(2).broadcast_to((128, 4, 4)),
                                        op=OP.mult)
                s4 = spool.tile([128, 4], fp32, tag="s4")
                nc.vector.tensor_reduce(out=s4[:],
                                        in_=wex[:].rearrange("p (r h) -> p h r", r=4),
                                        axis=mybir.AxisListType.X, op=OP.add)

                # o = v * s (broadcast over c), fused row-sum
                o_t = bpool.tile([128, 256], fp32, tag="o")
                sum_t = spool.tile([128, 1], fp32, tag="sum")
                nc.vector.scalar_tensor_tensor(
                    out=o_t[:].rearrange("p (h c) -> p h c", h=4),
                    in0=v_ps[:].rearrange("p (h c) -> p h c", h=4),
                    scalar=1.0, op0=OP.bypass,
                    in1=s4[:].unsqueeze(2).broadcast_to((128, 4, 64)),
                    op1=OP.mult, accum_out=sum_t[:])
                sq_t = bpool.tile([128, 256], fp32, tag="sq")
                ssq = spool.tile([128, 1], fp32, tag="ssq")
                nc.scalar.activation(sq_t[:], o_t[:], AF.Square, accum_out=ssq[:])
                mean = spool.tile([128, 1], fp32, tag="mean")
                nc.scalar.mul(mean[:], sum_t[:], 1.0 / 256.0)
                em2 = spool.tile([128, 1], fp32, tag="em2")
                nc.scalar.mul(em2[:], ssq[:], 1.0 / 256.0)
                m2 = spool.tile([128, 1], fp32, tag="m2")
                nc.vector.tensor_tensor(out=m2[:], in0=mean[:], in1=mean[:], op=OP.mult)
                varr = spool.tile([128, 1], fp32, tag="varr")
                nc.vector.scalar_tensor_tensor(out=varr[:], in0=em2[:], scalar=EPS,
                                               in1=m2[:], op0=OP.add,
                                               op1=OP.subtract)
                std = spool.tile([128, 1], fp32, tag="std")
                nc.scalar.sqrt(std[:], varr[:])
                rstd = spool.tile([128, 1], fp32, tag="rstd")
                nc.vector.reciprocal(rstd[:], std[:])
                nb = spool.tile([128, 1], fp32, tag="nb")
                nc.vector.scalar_tensor_tensor(out=nb[:], in0=mean[:], scalar=-1.0,
                                               in1=rstd[:], op0=OP.mult, op1=OP.mult)
                xh = bpool.tile([128, 256], fp32, tag="xh")
                nc.scalar.activation(xh[:], o_t[:], AF.Identity, scale=rstd[:], bias=nb[:])
                gz = bpool.tile([128, 256], fp32, tag="gz")
                nc.vector.tensor_tensor(out=gz[:], in0=xh[:], in1=gam_sb[:], op=OP.mult)
                zt = bpool.tile([128, 256], fp32, tag="zt")
                nc.vector.tensor_tensor(out=zt[:], in0=gz[:], in1=bet_sb[:], op=OP.add)
                yt = bpool.tile([128, 256], fp32, tag="yt")
                nc.scalar.activation(yt[:], zt[:], AF.Relu)
                nc.sync.dma_start(out=out[r0:r0 + 128, :], in_=yt[:])
    return nc



def _split_waits(bir_bytes):
    """Walrus on this stack only accepts one sync-wait per instruction.
    Split extra waits into standalone single-wait NoOps on the same
    engine queue (exact raw-bass semantics: in-order queue stalls)."""
    import orjson
    m = orjson.loads(bir_bytes)
    counter = [0]

    def proc(obj):
        if isinstance(obj, dict):
            for k, v in obj.items():
                if k == "instructions" and isinstance(v, list):
                    new = []
                    for ins in v:
                        si = ins.get("sync_info")
                        waits = (si or {}).get("on_wait") or []
                        lim = 0 if ins.get("opcode") == "ISA" else 1
                        if si and len(waits) > lim:
                            keep = waits[-lim:] if lim else []
                            for w in (waits[:-1] if lim else waits):
                                counter[0] += 1
                                new.append({
                                    "name": f"I-wsplit-{counter[0]}",
                                    "opcode": "EventSemaphore",
                                    "engine": ins.get("engine"),
                                    "ins": [], "outs": [],
                                    "debug": ins.get("debug"),
                                    "sync_info": {"on_update": [],
                                                  "on_wait": [w]},
                                })
                            si["on_wait"] = keep
                        new.append(ins)
                        proc(ins)
                    obj[k] = new
                else:
                    proc(v)
        elif isinstance(obj, list):
            for x in obj:
                proc(x)

    proc(m)
    return orjson.dumps(m)


def kernel(**inputs):
    global LAST_RESULT
    import os
    from concourse.bass_utils import run_bass_kernel_spmd

    feat = np.ascontiguousarray(np.asarray(inputs["feat"], dtype=np.float32))
    Wr = np.asarray(inputs["Wr"], dtype=np.float32)
    br = np.asarray(inputs["br"], dtype=np.float32)
    rl = np.asarray(inputs["rel_attn_l"], dtype=np.float32)
    rr = np.asarray(inputs["rel_attn_r"], dtype=np.float32)
    g = np.asarray(inputs["ln_gamma"], dtype=np.float32)
    b = np.asarray(inputs["ln_beta"], dtype=np.float32)

    # per-node "has incoming edge" masks (graph structure -> node sharding prep)
    mask = np.ones((N, 4), np.float32)
    for m in range(M):
        dst = np.asarray(inputs[f"dst{m}"])
        mask[:, m] = np.bincount(dst, minlength=N) > 0

    # fold rel_attn into the weight matrix:  vl = feat @ (Wr @ RLbd) + br@RLbd
    rl_bd = np.zeros((256, 4), np.float32)
    rr_bd = np.zeros((256, 4), np.float32)
    for h in range(H):
        rl_bd[h * C:(h + 1) * C, h] = rl[h]
        rr_bd[h * C:(h + 1) * C, h] = rr[h]
    A = np.concatenate([Wr @ rl_bd, Wr @ rr_bd], axis=1)          # [256, 8]
    abias = np.concatenate([br @ rl_bd, br @ rr_bd])              # [8]

    const = np.zeros((128, 1628), np.float32)
    const[:, 0:256] = Wr[0:128]
    const[:, 256:512] = Wr[128:256]
    const[:, 512:520] = A[0:128]
    const[:, 520:528] = A[128:256]
    const[:, 528:784] = g
    const[:, 784:1040] = b
    const[0, 1040:1296] = br
    const[0, 1296:1304] = abias
    const[0, 1304:1432] = 1.0

    key = "nc"
    if key not in _CACHE:
        nc0 = _build()
        _orig = nc0.to_json_bytes
        nc0.to_json_bytes = lambda: _split_waits(_orig())
        _CACHE[key] = nc0
    nc = _CACHE[key]

    in_maps = []
    for s in range(NCORES):
        fs = np.zeros((RPAD, 256), np.float32)
        fs[:RPC] = feat[s * RPC:(s + 1) * RPC]
        # featT[p, k, j] = fs[j, k*128 + p]
        ftT = np.ascontiguousarray(fs.T.reshape(2, 128, RPAD).transpose(1, 0, 2))
        mk = np.ones((RPAD, 4), np.float32)
        mk[:RPC] = mask[s * RPC:(s + 1) * RPC]
        cs = const.copy()
        cs[:, 1432:1628] = mk.reshape(NT, 128, 4).transpose(1, 0, 2).reshape(128, NT * 4)
        in_maps.append({"featT": ftT, "constd": cs})

    trace = bool(int(os.environ.get("KERNEL_TRACE", "0")))
    res = run_bass_kernel_spmd(nc, in_maps, list(range(NCORES)), trace=trace)
    LAST_RESULT = res
    outs = [res.results[s]["out"][:RPC] for s in range(NCORES)]
    return np.concatenate(outs, axis=0)



# revision 18
# speedup vs baseline: 5.6915x; 5.6915x over previous
"""LATTE GNN forward on 8 Trainium2 NeuronCores (v2, bf16 pipeline).

Math collapse (see reference): per-edge message is v[dst], softmax weights
sum to 1, so edge aggregation = v * mask. Masks are binary, so all mask=1
relations share the logit lrelu(vl+vr) and the relation-softmax collapses:
    v       = feat @ Wr                      (bias zero on fast path)
    vl,vr,vs= per-(n,h) projections of v     (extra matmul columns)
    eA      = exp(lrelu(vl+vr));  denA = sum_h eA
    s4[n,h] = eA * cnt[n]/denA    (cnt = 1 + #relations with an in-edge)
    mu      = sum_h s4*vs / 256;  mc = mu/s4
    w       = v - mc              (o - mu = s4*w exactly)
    var     = sum_h s4^2 * sum_c w^2 / 256
    rstd    = exp(-0.5*ln(var+eps))
    y       = max(w,0) * (s4*rstd)           (gamma=1, beta=0 fast path)
v is stored (c,h)-major (d' = c*4+h) so per-(tile,h) broadcast operands are
innermost-stride-1 -> bf16 2x DVE mode. Host un-permutes columns at the end.
Node-sharded 8 cores x 6250 rows (padded 6272 = 49*128).
"""

import numpy as np
import ml_dtypes

N, D, H, C, M = 50000, 256, 4, 64, 3
NCORES = 8
RPC = N // NCORES          # 6250
NT = 49
RPAD = NT * 128            # 6272
EPS = 1e-5
BF = ml_dtypes.bfloat16

# pipeline structure
GROUPS = [(0, 12), (12, 24), (24, 36), (36, 49)]   # tile ranges
CH = 2                                             # tiles per PSUM chunk
NCHUNK = 25                                        # 24*2 + 1
CHUNK_BUFS = 4                                     # PSUM: 4 chunk banks + 4 smalls

_CACHE = {}
LAST_RESULT = None


def _build(fast=True):
    import concourse.bass as bass
    import concourse.mybir as mybir
    from concourse.tile import TileContext

    fp32 = mybir.dt.float32
    bf16 = mybir.dt.bfloat16
    AF = mybir.ActivationFunctionType
    OP = mybir.AluOpType

    nc = bass.Bass()
    ftd = nc.declare_dram_parameter("ftT", [128, 2, RPAD], bf16, isOutput=False)
    wcd = nc.declare_dram_parameter("wcst", [128, 2, 268], bf16, isOutput=False)
    cntd = nc.declare_dram_parameter("cnt", [128, NT], fp32, isOutput=False)
    outd = nc.declare_dram_parameter("out", [128, NT * 256], bf16, isOutput=True)
    if not fast:
        # gb[:,0,0:256]=gamma (c,h)-major, gb[:,1,0:256]=beta,
        # gb[:,2,:]=bias row [br' | abias] (c,h)-major, applied via 1-row matmul
        gbd = nc.declare_dram_parameter("gb", [128, 3, 268], fp32, isOutput=False)

    with TileContext(nc) as tc:
        with (
            tc.tile_pool(name="const", bufs=1) as cpool,
            tc.tile_pool(name="sb", bufs=1) as sbp,
            tc.tile_pool(name="sm", bufs=1) as smp,
            tc.tile_pool(name="pv", bufs=3, space="PSUM") as pvp,
            tc.tile_pool(name="psm", bufs=1, space="PSUM") as psp,
        ):
            # ---- act table warmup (one set: natural_log_exp_and_others) ----
            warm = cpool.tile([128, 8], fp32, tag="warm")
            nc.gpsimd.memset(warm[:], 0.0)
            warm2 = cpool.tile([128, 8], fp32, tag="warm2")
            nc.scalar.activation(warm2[:], warm[:], AF.Exp)
            nc.scalar.activation(warm2[:], warm2[:], AF.Ln)
            eps_sb = cpool.tile([128, 1], fp32, tag="eps")
            nc.gpsimd.memset(eps_sb[:], EPS)

            # ---- constants ----
            w_sb = cpool.tile([128, 2, 268], bf16, tag="wc")
            nc.gpsimd.dma_start(out=w_sb[:], in_=wcd[:])
            cnt_sb = cpool.tile([128, NT], fp32, tag="cnt")
            nc.gpsimd.dma_start(out=cnt_sb[:], in_=cntd[:])
            if not fast:
                gb_sb = cpool.tile([128, 3, 268], fp32, tag="gb")
                nc.gpsimd.dma_start(out=gb_sb[:], in_=gbd[:])
                ones_sb = cpool.tile([128, 128], bf16, tag="ones")
                nc.gpsimd.memset(ones_sb[:], 1.0)
                brow = cpool.tile([128, 268], bf16, tag="brow")
                nc.vector.tensor_copy(out=brow[:], in_=gb_sb[:, 2, :])

            # ---- feature tiles (stationary operands), 7 DMA pieces ----
            ft = sbp.tile([128, 2, RPAD], bf16, tag="ft")
            PIECE = 7 * 128
            for i in range(7):
                nc.sync.dma_start(out=ft[:, :, i * PIECE:(i + 1) * PIECE],
                                  in_=ftd[:, :, i * PIECE:(i + 1) * PIECE])

            # ---- big SBUF buffers ----
            v_all = sbp.tile([128, NT, 256], bf16, tag="v")
            w_all = sbp.tile([128, NT, 256], bf16, tag="w")
            q_all = sbp.tile([128, NT, 256], bf16, tag="q")    # w^2 / tree scratch
            m_all = sbp.tile([128, NT, 256], bf16, tag="m")    # relu(w)
            y_all = sbp.tile([128, NT, 256], bf16, tag="y")

            # ---- small per-(t,h) tensors ----
            sm_all = smp.tile([128, NT, 12], fp32, tag="sml")  # vl|vr|vs
            lvr = smp.tile([128, NT, 4], fp32, tag="lvr")
            eA = smp.tile([128, NT, 4], fp32, tag="eA")
            den = smp.tile([128, NT], fp32, tag="den")
            rq = smp.tile([128, NT], fp32, tag="rq")
            s4 = smp.tile([128, NT, 4], fp32, tag="s4")
            mus = smp.tile([128, NT, 4], fp32, tag="mus")
            mean = smp.tile([128, NT], fp32, tag="mean")
            rs4 = smp.tile([128, NT, 4], fp32, tag="rs4")
            mcn = smp.tile([128, NT, 4], bf16, tag="mcn")
            s4sq = smp.tile([128, NT, 4], fp32, tag="s4sq")
            prod = smp.tile([128, NT, 4], fp32, tag="prod")
            o2 = smp.tile([128, NT], fp32, tag="o2")
            rstd = smp.tile([128, NT], fp32, tag="rstd")
            spp = smp.tile([128, NT, 4], bf16, tag="spp")

            # ---- PSUM ----
            # one smalls bank per group (PE-W and DVE-R of the same PSUM bank
            # must never overlap in time -> reader waits for the whole tile)
            smS = [psp.tile([128, (GROUPS[g][1] - GROUPS[g][0]) * 12], fp32,
                            tag=f"sm{g}", name=f"smS{g}") for g in range(4)]
            vch = [None] * NCHUNK

            def pe_chunk(c):
                for t in range(c * CH, min((c + 1) * CH, NT)):
                    sl = t % CH
                    if sl == 0 or vch[c] is None:
                        vch[c] = pvp.tile([128, CH * 256], fp32, tag="vch",
                                          name=f"vch{c}", bufs=CHUNK_BUFS)
                    vout = vch[c][:, sl * 256:(sl + 1) * 256]
                    g = next(i for i, (a, b) in enumerate(GROUPS) if a <= t < b)
                    smt = smS[g][:, (t - GROUPS[g][0]) * 12:
                                 (t - GROUPS[g][0] + 1) * 12]
                    f0 = ft[:, 0, t * 128:(t + 1) * 128]
                    f1 = ft[:, 1, t * 128:(t + 1) * 128]
                    last = fast  # general path appends bias matmuls
                    nc.tensor.matmul(vout, f0, w_sb[:, 0, 0:256],
                                     start=True, stop=False)
                    nc.tensor.matmul(smt, f0, w_sb[:, 0, 256:268],
                                     start=True, stop=False)
                    nc.tensor.matmul(vout, f1, w_sb[:, 1, 0:256],
                                     start=False, stop=last)
                    nc.tensor.matmul(smt, f1, w_sb[:, 1, 256:268],
                                     start=False, stop=last)
                    if not fast:
                        nc.tensor.matmul(vout, ones_sb[0:1, :],
                                         brow[0:1, 0:256],
                                         start=False, stop=True)
                        nc.tensor.matmul(smt, ones_sb[0:1, :],
                                         brow[0:1, 256:268],
                                         start=False, stop=True)

            def evac(c):
                if c == NCHUNK - 1 and NT % CH:
                    nc.vector.tensor_copy(
                        out=v_all[:, c * CH, :],
                        in_=vch[c][:, 0:256])
                else:
                    nc.vector.tensor_copy(
                        out=v_all[:, c * CH:(c + 1) * CH, :],
                        in_=vch[c][:].rearrange("p (t d) -> p t d", d=256))

            # ============ grouped, software-pipelined back half ============
            # Emission is in DATAFLOW order (tile framework derives deps from
            # program order); pipelining comes from interleaving group stages:
            # S1(0) S1(1) S2(0) S1(2) S2(1) S1(3) S2(2) S2(3).

            def stage1(g):
                t0, t1 = GROUPS[g]
                s = slice(t0, t1)
                nt = t1 - t0
                # -- smalls PSUM -> SBUF (whole-bank read: PE done with it) --
                nc.vector.tensor_copy(
                    out=sm_all[:, s, :],
                    in_=smS[g][:].rearrange("p (t c) -> p t c", c=12))
                # -- s-chain --
                nc.vector.tensor_tensor(out=lvr[:, s, :], in0=sm_all[:, s, 0:4],
                                        in1=sm_all[:, s, 4:8], op=OP.add)
                nc.vector.scalar_tensor_tensor(
                    out=lvr[:, s, :], in0=lvr[:, s, :], scalar=0.2,
                    in1=lvr[:, s, :], op0=OP.mult, op1=OP.max)
                nc.scalar.activation(eA[:, s, :], lvr[:, s, :], AF.Exp)
                nc.vector.tensor_reduce(out=den[:, s], in_=eA[:, s, :],
                                        axis=mybir.AxisListType.X, op=OP.add)
                nc.scalar.activation(den[:, s], den[:, s], AF.Ln)
                nc.scalar.activation(rq[:, s], den[:, s], AF.Exp, scale=-1.0)
                nc.vector.tensor_tensor(out=rq[:, s], in0=cnt_sb[:, s],
                                        in1=rq[:, s], op=OP.mult)
                nc.vector.tensor_tensor(
                    out=s4[:, s, :], in0=eA[:, s, :],
                    in1=rq[:, s].unsqueeze(2).broadcast_to((128, nt, 4)),
                    op=OP.mult)
                nc.vector.tensor_tensor(out=mus[:, s, :], in0=s4[:, s, :],
                                        in1=sm_all[:, s, 8:12], op=OP.mult)
                nc.vector.tensor_reduce(out=mean[:, s], in_=mus[:, s, :],
                                        axis=mybir.AxisListType.X, op=OP.add)
                nc.scalar.mul(mean[:, s], mean[:, s], 1.0 / 256.0)
                nc.scalar.activation(rs4[:, s, :], s4[:, s, :], AF.Ln)
                nc.scalar.activation(rs4[:, s, :], rs4[:, s, :], AF.Exp,
                                     scale=-1.0)
                nc.scalar.activation(s4sq[:, s, :], s4[:, s, :], AF.Square)
                # mcneg = -mean * (1/s4)
                nc.vector.scalar_tensor_tensor(
                    out=mcn[:, s, :], in0=rs4[:, s, :], scalar=-1.0,
                    in1=mean[:, s].unsqueeze(2).broadcast_to((128, nt, 4)),
                    op0=OP.mult, op1=OP.mult)
                # -- B2: w = v + mcneg_bcast --
                v4 = v_all[:, t0:t1, :].rearrange("p t (c h) -> p t c h", h=4)
                w4 = w_all[:, t0:t1, :].rearrange("p t (c h) -> p t c h", h=4)
                nc.vector.tensor_tensor(
                    out=w4, in0=v4,
                    in1=mcn[:, s, :].unsqueeze(2).broadcast_to((128, nt, 64, 4)),
                    op=OP.add)
                # -- B3: q = w^2 (Act) --
                nc.scalar.activation(q_all[:, t0:t1, :], w_all[:, t0:t1, :],
                                     AF.Square)

            def stage2(g):
                t0, t1 = GROUPS[g]
                s = slice(t0, t1)
                nt = t1 - t0
                # -- add-tree over c (in place, bf16 2x) --
                q4 = q_all[:, t0:t1, :].rearrange("p t (c h) -> p t c h", h=4)
                cc = 64
                while cc > 1:
                    hh = cc // 2
                    nc.vector.tensor_tensor(out=q4[:, :, 0:hh, :],
                                            in0=q4[:, :, 0:hh, :],
                                            in1=q4[:, :, hh:cc, :], op=OP.add)
                    cc = hh
                # -- o2 = sum_h s4^2 * wsq4 ; rstd = exp(-.5 ln(o2/256+eps)) --
                nc.vector.tensor_tensor(out=prod[:, s, :], in0=s4sq[:, s, :],
                                        in1=q_all[:, s, 0:4], op=OP.mult)
                nc.vector.tensor_reduce(out=o2[:, s], in_=prod[:, s, :],
                                        axis=mybir.AxisListType.X, op=OP.add)
                nc.scalar.activation(rstd[:, s], o2[:, s], AF.Ln,
                                     scale=1.0 / 256.0, bias=eps_sb[:])
                nc.scalar.activation(rstd[:, s], rstd[:, s], AF.Exp, scale=-0.5)
                # -- s'' = s4*rstd ; y = max(w,0)*s''_bcast --
                nc.vector.scalar_tensor_tensor(
                    out=spp[:, s, :], in0=s4[:, s, :], scalar=1.0,
                    in1=rstd[:, s].unsqueeze(2).broadcast_to((128, nt, 4)),
                    op0=OP.bypass, op1=OP.mult)
                m4 = m_all[:, t0:t1, :].rearrange("p t (c h) -> p t c h", h=4)
                y4 = y_all[:, t0:t1, :].rearrange("p t (c h) -> p t c h", h=4)
                if fast:
                    nc.vector.tensor_scalar_max(m_all[:, t0:t1, :],
                                                w_all[:, t0:t1, :], 0.0)
                    nc.vector.tensor_tensor(
                        out=y4, in0=m4,
                        in1=spp[:, s, :].unsqueeze(2).broadcast_to(
                            (128, nt, 64, 4)),
                        op=OP.mult)
                else:
                    w4 = w_all[:, t0:t1, :].rearrange("p t (c h) -> p t c h", h=4)
                    nc.vector.tensor_tensor(
                        out=y4, in0=w4,
                        in1=spp[:, s, :].unsqueeze(2).broadcast_to(
                            (128, nt, 64, 4)),
                        op=OP.mult)
                    zf = y_all[:, t0:t1, :]
                    nc.vector.tensor_tensor(
                        out=zf, in0=zf,
                        in1=gb_sb[:, 0, :].unsqueeze(1).broadcast_to(
                            (128, nt, 256)), op=OP.mult)
                    nc.vector.tensor_tensor(
                        out=zf, in0=zf,
                        in1=gb_sb[:, 1, :].unsqueeze(1).broadcast_to(
                            (128, nt, 256)), op=OP.add)
                    nc.vector.tensor_scalar_max(zf, zf, 0.0)
                nc.sync.dma_start(out=outd[:, t0 * 256:t1 * 256],
                                  in_=y_all[:, t0:t1, :])

            # PE/evac interleaved so chunk slots are reused only after their
            # reader is emitted (program order = dependency order); stages
            # injected as soon as their group's chunks are evacuated.
            # group -> last chunk: g0: c5, g1: c11, g2: c17, g3: c24
            inject = {9: [lambda: stage1(0)],
                      15: [lambda: stage1(1), lambda: stage2(0)],
                      21: [lambda: stage1(2), lambda: stage2(1)]}
            for c in range(NCHUNK):
                if c >= CHUNK_BUFS:
                    evac(c - CHUNK_BUFS)
                pe_chunk(c)
                for fn in inject.get(c, []):
                    fn()
            for c in range(NCHUNK - CHUNK_BUFS, NCHUNK):
                evac(c)
            stage1(3)
            stage2(2)
            stage2(3)
    return nc


def _split_waits(bir_bytes):
    """Walrus on this stack only accepts one sync-wait per instruction.
    Split extra waits into standalone single-wait NoOps on the same
    engine queue (exact raw-bass semantics: in-order queue stalls)."""
    import orjson
    m = orjson.loads(bir_bytes)
    counter = [0]

    def proc(obj):
        if isinstance(obj, dict):
            for k, v in obj.items():
                if k == "instructions" and isinstance(v, list):
                    new = []
                    for ins in v:
                        si = ins.get("sync_info")
                        waits = (si or {}).get("on_wait") or []
                        lim = 0 if ins.get("opcode") == "ISA" else 1
                        if si and len(waits) > lim:
                            keep = waits[-lim:] if lim else []
                            for w in (waits[:-1] if lim else waits):
                                counter[0] += 1
                                new.append({
                                    "name": f"I-wsplit-{counter[0]}",
                                    "opcode": "EventSemaphore",
                                    "engine": ins.get("engine"),
                                    "ins": [], "outs": [],
                                    "debug": ins.get("debug"),
                                    "sync_info": {"on_update": [],
                                                  "on_wait": [w]},
                                })
                            si["on_wait"] = keep
                        new.append(ins)
                        proc(ins)
                    obj[k] = new
                else:
                    proc(v)
        elif isinstance(obj, list):
            for x in obj:
                proc(x)

    proc(m)
    return orjson.dumps(m)


def kernel(**inputs):
    global LAST_RESULT
    import os
    from concourse.bass_utils import run_bass_kernel_spmd

    feat = np.ascontiguousarray(np.asarray(inputs["feat"], dtype=np.float32))
    Wr = np.asarray(inputs["Wr"], dtype=np.float32)
    br = np.asarray(inputs["br"], dtype=np.float32)
    rl = np.asarray(inputs["rel_attn_l"], dtype=np.float32)
    rr = np.asarray(inputs["rel_attn_r"], dtype=np.float32)
    g = np.asarray(inputs["ln_gamma"], dtype=np.float32)
    b = np.asarray(inputs["ln_beta"], dtype=np.float32)

    fast = (not br.any()) and (not b.any()) and np.all(g == 1.0)

    # cnt[n] = 1 + #relations with >=1 incoming edge at n
    cnt = np.ones(N, np.float32)
    for m in range(M):
        dst = np.asarray(inputs[f"dst{m}"])
        cnt += (np.bincount(dst, minlength=N) > 0)

    # weight prep: (c,h)-major columns + smalls columns [vl|vr|vs]
    Wr3 = Wr.reshape(256, H, C)
    Wp = Wr3.transpose(0, 2, 1).reshape(256, 256)        # d' = c*4+h
    AL = np.einsum('khc,hc->kh', Wr3, rl)                # [256,4]
    AR = np.einsum('khc,hc->kh', Wr3, rr)
    AS = Wr3.sum(2)                                      # [256,4]
    Wfull = np.concatenate([Wp, AL, AR, AS], axis=1)     # [256, 268]
    wcst = np.ascontiguousarray(Wfull.reshape(2, 128, 268).transpose(1, 0, 2)
                                ).astype(BF)             # [128, 2, 268]

    key = ("v2", fast)
    if key not in _CACHE:
        nc0 = _build(fast=fast)
        _orig = nc0.to_json_bytes
        nc0.to_json_bytes = lambda: _split_waits(_orig())
        _CACHE[key] = nc0
    nc = _CACHE[key]

    in_maps = []
    for s in range(NCORES):
        fs = np.zeros((RPAD, 256), np.float32)
        fs[:RPC] = feat[s * RPC:(s + 1) * RPC]
        ftT = np.ascontiguousarray(
            fs.T.reshape(2, 128, RPAD).transpose(1, 0, 2)).astype(BF)
        cs = np.full(RPAD, 4.0, np.float32)
        cs[:RPC] = cnt[s * RPC:(s + 1) * RPC]
        cnt_pt = np.ascontiguousarray(cs.reshape(NT, 128).T)  # [128, NT]
        im = {"ftT": ftT, "wcst": wcst, "cnt": cnt_pt}
        if not fast:
            gp = g.reshape(H, C).T.reshape(256)              # (c,h)-major
            bp = b.reshape(H, C).T.reshape(256)
            brp = br.reshape(H, C).T.reshape(256)
            br3 = br.reshape(H, C)
            abias = np.concatenate([(br3 * rl).sum(1), (br3 * rr).sum(1),
                                    br3.sum(1)])             # [12]
            gb = np.zeros((128, 3, 268), np.float32)
            gb[:, 0, 0:256] = gp
            gb[:, 1, 0:256] = bp
            gb[:, 2, 0:256] = brp
            gb[:, 2, 256:268] = abias
            im["gb"] = gb
        in_maps.append(im)

    trace = bool(int(os.environ.get("KERNEL_TRACE", "0")))
    res = run_bass_kernel_spmd(nc, in_maps, list(range(NCORES)), trace=trace)
    LAST_RESULT = res

    outs = []
    for s in range(NCORES):
        y = np.asarray(res.results[s]["out"]).astype(np.float32)
        y = y.reshape(128, NT, 256).transpose(1, 0, 2).reshape(RPAD, 256)[:RPC]
        # un-permute columns: stored d' = c*4+h -> standard d = h*64+c
        y = y.reshape(-1, C, H).transpose(0, 2, 1).reshape(-1, 256)
        outs.append(y)
    return np.concatenate(outs, axis=0)
